# revision 8
# baseline (speedup 1.0000x reference)
"""Dilated multi-head attention (nn_DilatedMHA) on 8 trn2 NeuronCores.

Math (reference restructured):
  qkv = x @ Wqkv.T                      [b, n, 3, h, d]   b=2, n=8192, h=12, d=64
  Coupling structure: position i only attends within its mod-2048 class
  {p, p+2048, p+4096, p+6144} (p = i % 2048).  Per group p and head:
    r=1 branch: full 4x4 softmax attention over the 4 slots.
    r=2 branch (p even): 2x2 attention among same-parity slots.
    r=4 branch (p % 4 == 0): adds v.
  out is then normalized by sum over the whole sequence per (b, h*d) channel
  and projected by Wout.

Sharding: core c <- batch c//4, groups p in [(c%4)*512, (c%4)*512+512).
Each core: QKV projection (fp32 matmul), attention on DVE (fp32),
channel-sum AllReduce over the 4 cores of the batch, out-projection
(fp32r matmul, full PE rate) with 1/s folded into Wout.

Precision notes: the normalization denominator min |s| is ~0.018 while the
8192 summands are ~0.5, so upstream per-element noise is amplified ~1000x
into the output.  bf16/tf32 projections fail outright (measured 1.27 / 0.65
abs-max relative error); QKV must run at fp32 matmul precision and all
attention arithmetic in fp32 (DVE is fp32 internally, ACT exp is ~2 ULP).
The channel sums are taken from the exact fp32 PSUM transpose tiles; only
the out-projection operands (downstream of the division) are fp32r.
"""

import sys

if "/opt/trn_rl_repo" not in sys.path:
    sys.path.insert(0, "/opt/trn_rl_repo")

import numpy as np

EMBED = 768
HEADS = 12
HD = 64
B = 2
N = 8192
NCORES = 8
GPC = 512           # groups per core
NBLK = 4            # blocks of 128 groups per core
NCHUNK = 16         # row chunks of 128 per core (blk, j)
KO = 6              # embed // 128

_COMPILED = {}


def _build_program():
    import concourse.mybir as mybir
    import concourse.tile as tile
    from concourse import bacc

    F32 = mybir.dt.float32
    F32R = mybir.dt.float32r
    AX = mybir.AxisListType
    OP = mybir.AluOpType
    ACTF = mybir.ActivationFunctionType

    nc = bacc.Bacc("TRN2", target_bir_lowering=False, debug=False, num_devices=NCORES)

    # --- DRAM I/O ---------------------------------------------------------
    xc_d = nc.dram_tensor("xc", [NCHUNK, 128, KO, 128], F32, kind="ExternalInput")
    wq_d = nc.dram_tensor("wq", [128, KO, 3 * EMBED], F32, kind="ExternalInput")
    wo_d = nc.dram_tensor("wo", [128, KO, EMBED], F32, kind="ExternalInput")
    m2_d = nc.dram_tensor("m2", [128, 1], F32, kind="ExternalInput")
    m4_d = nc.dram_tensor("m4", [128, 1], F32, kind="ExternalInput")
    id_d = nc.dram_tensor("ident", [128, 128], F32, kind="ExternalInput")
    y_d = nc.dram_tensor("y", [4 * GPC, EMBED], F32, kind="ExternalOutput")

    with tile.TileContext(nc) as tc:
        with (
            tc.tile_pool(name="const", bufs=1) as constp,
            tc.tile_pool(name="oT", bufs=4) as oTp,
            tc.tile_pool(name="mm", bufs=6, space="PSUM") as mmp,
            tc.tile_pool(name="tp", bufs=2, space="PSUM") as tpp,
            tc.tile_pool(name="dram", bufs=2, space="DRAM") as dramp,
        ):
            # --- long-lived SBUF -----------------------------------------
            wq_sb = constp.tile([128, KO, 3 * EMBED], F32)
            nc.sync.dma_start(wq_sb[:], wq_d[:])
            wo_sb = constp.tile([128, KO, EMBED], F32)
            nc.sync.dma_start(wo_sb[:], wo_d[:])
            m2_sb = constp.tile([128, 1], F32)
            nc.sync.dma_start(m2_sb[:], m2_d[:])
            m4_sb = constp.tile([128, 1], F32)
            nc.sync.dma_start(m4_sb[:], m4_d[:])
            id_sb = constp.tile([128, 128], F32)
            nc.sync.dma_start(id_sb[:], id_d[:])
            s_acc = constp.tile([128, KO], F32)
            nc.vector.memset(s_acc[:], 0.0)

            oT_blocks = []

            def hview(ap2d):
                # [128, 768] -> [128, 12, 64]
                return ap2d.rearrange("p (h d) -> p h d", d=HD)

            # =============== Phase A: QKV + attention ====================
            with (
                tc.tile_pool(name="xc", bufs=2) as xcp,
                tc.tile_pool(name="qkv", bufs=5) as qkvp,
                tc.tile_pool(name="oacc", bufs=2) as oaccp,
                tc.tile_pool(name="att", bufs=1) as attp,
                tc.tile_pool(name="prod", bufs=1) as prodp,
                tc.tile_pool(name="sp", bufs=2) as spp,
            ):
                for blk in range(NBLK):
                    qkv_tiles = []
                    for j in range(4):
                        chunk = blk * 4 + j
                        xc_sb = xcp.tile([128, KO, 128], F32, tag="xc")
                        nc.sync.dma_start(xc_sb[:], xc_d[chunk])
                        q_sb = qkvp.tile([128, 3 * EMBED], F32, tag="qkv")
                        # 6 output column tiles of 384; accumulate over ko.
                        for n in range(6):
                            ps = mmp.tile([128, 384], F32, tag="mm")
                            for ko in range(KO):
                                nc.tensor.matmul(
                                    ps[:],
                                    lhsT=xc_sb[:, ko, :],
                                    rhs=wq_sb[:, ko, n * 384:(n + 1) * 384],
                                    start=(ko == 0),
                                    stop=(ko == KO - 1),
                                )
                            nc.scalar.copy(q_sb[:, n * 384:(n + 1) * 384], ps[:])
                        qkv_tiles.append(q_sb)

                    Q = [hview(qkv_tiles[j][:, 0:EMBED]) for j in range(4)]
                    K = [hview(qkv_tiles[j][:, EMBED:2 * EMBED]) for j in range(4)]
                    V = [hview(qkv_tiles[j][:, 2 * EMBED:3 * EMBED]) for j in range(4)]

                    # --- scores -> exp'd scores E[p, j, h, j'] ---------------
                    S = attp.tile([128, 4, HEADS, 4], F32, tag="S")
                    for j in range(4):
                        for jp in range(4):
                            pr = prodp.tile([128, HEADS, HD], F32, tag="prod")
                            nc.vector.tensor_mul(pr[:], Q[j], K[jp])
                            nc.vector.reduce_sum(S[:, j, :, jp], pr[:], axis=AX.X)
                    E = attp.tile([128, 4, HEADS, 4], F32, tag="E")
                    nc.scalar.activation(E[:], S[:], ACTF.Exp, scale=0.125)

                    # --- attention weights Wt --------------------------------
                    Z1 = attp.tile([128, 4, HEADS], F32, tag="Z1")
                    nc.vector.reduce_sum(Z1[:], E[:], axis=AX.X)
                    R1 = attp.tile([128, 4, HEADS], F32, tag="R1")
                    nc.vector.reciprocal(R1[:], Z1[:])
                    Z2 = attp.tile([128, 4, HEADS, 2], F32, tag="Z2")
                    nc.vector.tensor_add(Z2[:], E[:, :, :, 0:2], E[:, :, :, 2:4])
                    R2 = attp.tile([128, 4, HEADS, 2], F32, tag="R2")
                    nc.vector.reciprocal(R2[:], Z2[:])

                    W1 = attp.tile([128, 4, HEADS, 4], F32, tag="W1")
                    nc.vector.tensor_mul(
                        W1[:], E[:], R1[:, :, :, None].to_broadcast((128, 4, HEADS, 4))
                    )
                    W2 = attp.tile([128, 4, HEADS, 4], F32, tag="W2")
                    nc.vector.memset(W2[:], 0.0)
                    for par in (0, 1):
                        nc.vector.tensor_mul(
                            W2[:, par::2, :, par::2],
                            E[:, par::2, :, par::2],
                            R2[:, par::2, :, par:par + 1].to_broadcast(
                                (128, 2, HEADS, 2)
                            ),
                        )
                    Wt = attp.tile([128, 4, HEADS, 4], F32, tag="Wt")
                    nc.vector.scalar_tensor_tensor(
                        Wt[:], W2[:], m2_sb[:, 0:1], W1[:], OP.mult, OP.add
                    )
                    for j in range(4):
                        nc.vector.tensor_scalar_add(
                            Wt[:, j, :, j:j + 1], Wt[:, j, :, j:j + 1], m4_sb[:, 0:1]
                        )

                    # --- AV --------------------------------------------------
                    oacc = oaccp.tile([128, 4, EMBED], F32, tag="oacc")
                    for j in range(4):
                        oj = hview(oacc[:, j, :])
                        for jp in range(4):
                            wb = Wt[:, j, :, jp:jp + 1].to_broadcast((128, HEADS, HD))
                            if jp == 0:
                                nc.vector.tensor_mul(oj, V[jp], wb)
                            else:
                                pr = prodp.tile([128, HEADS, HD], F32, tag="prod")
                                nc.vector.tensor_mul(pr[:], V[jp], wb)
                                nc.vector.tensor_add(oj, oj, pr[:])

                    # --- transpose to oT[hd, rows] + channel sums -----------
                    # oT is fp32r (rounded by the ACT copy) for the fp32r
                    # out-projection; the channel sums come from the exact
                    # fp32 PSUM tiles.
                    oT = oTp.tile([128, KO, 4 * 128], F32R, tag="oT")
                    for j in range(4):
                        for ko in range(KO):
                            pt = tpp.tile([128, 128], F32, tag="tp")
                            nc.tensor.transpose(
                                pt[:], oacc[:, j, ko * 128:(ko + 1) * 128], id_sb[:]
                            )
                            nc.scalar.copy(oT[:, ko, j * 128:(j + 1) * 128], pt[:])
                            stmp = spp.tile([128, 1], F32, tag="sp")
                            nc.vector.reduce_sum(stmp[:], pt[:], axis=AX.X)
                            nc.vector.tensor_add(
                                s_acc[:, ko:ko + 1], s_acc[:, ko:ko + 1], stmp[:]
                            )
                    oT_blocks.append(oT)

            # =============== AllReduce of channel sums ===================
            cc_in = dramp.tile([128, KO], F32)
            cc_out = dramp.tile([128, KO], F32)
            nc.gpsimd.dma_start(cc_in[:], s_acc[:])
            nc.gpsimd.collective_compute(
                "AllReduce",
                OP.add,
                replica_groups=[[0, 1, 2, 3], [4, 5, 6, 7]],
                ins=[cc_in[:].opt()],
                outs=[cc_out[:].opt()],
            )
            s_tot = constp.tile([128, KO], F32)
            nc.gpsimd.dma_start(s_tot[:], cc_out[:])
            r_sb = constp.tile([128, KO], F32)
            nc.vector.reciprocal(r_sb[:], s_tot[:])

            # =============== Phase B: out-projection =====================
            with (
                tc.tile_pool(name="ws", bufs=1) as wsp,
                tc.tile_pool(name="fin", bufs=3) as finp,
            ):
                # fold 1/s into Wout rows (per-partition scalar per ko);
                # the DVE write rounds to fp32r as the verifier requires.
                ws_sb = wsp.tile([128, KO, EMBED], F32R)
                for ko in range(KO):
                    nc.vector.tensor_scalar_mul(
                        ws_sb[:, ko, :], wo_sb[:, ko, :], r_sb[:, ko:ko + 1]
                    )

                for blk in range(NBLK):
                    oT = oT_blocks[blk]
                    for rc in range(4):
                        for half in range(2):
                            pf = mmp.tile([128, 384], F32, tag="mm")
                            for ko in range(KO):
                                nc.tensor.matmul(
                                    pf[:],
                                    lhsT=oT[:, ko, rc * 128:(rc + 1) * 128],
                                    rhs=ws_sb[:, ko, half * 384:(half + 1) * 384],
                                    start=(ko == 0),
                                    stop=(ko == KO - 1),
                                )
                            fin = finp.tile([128, 384], F32, tag="fin")
                            nc.scalar.copy(fin[:], pf[:])
                            rows = blk * 512 + rc * 128
                            nc.sync.dma_start(
                                y_d[rows:rows + 128, half * 384:(half + 1) * 384],
                                fin[:],
                            )

    nc.finalize()
    return nc


def _host_shard(x, Wqkv, Wout):
    """Build per-core input maps."""
    x = np.ascontiguousarray(np.asarray(x, dtype=np.float32))
    Wqkv = np.asarray(Wqkv, dtype=np.float32)
    Wout = np.asarray(Wout, dtype=np.float32)

    wq = np.ascontiguousarray(
        Wqkv.T.reshape(KO, 128, 3 * EMBED).transpose(1, 0, 2)
    )
    wo = np.ascontiguousarray(Wout.T.reshape(KO, 128, EMBED).transpose(1, 0, 2))
    m2 = (np.arange(128) % 2 == 0).astype(np.float32).reshape(128, 1)
    m4 = (np.arange(128) % 4 == 0).astype(np.float32).reshape(128, 1)
    ident = np.eye(128, dtype=np.float32)

    in_maps = []
    for c in range(NCORES):
        bc, q = divmod(c, 4)
        xb = x[bc].reshape(4, 4, 4, 128, EMBED)  # [j, q, blk, g, e]
        mine = xb[:, q]                          # [j, blk, g, e]
        t = np.ascontiguousarray(mine.transpose(1, 0, 2, 3)).reshape(
            NCHUNK, 128, EMBED
        )
        xc = np.ascontiguousarray(
            t.reshape(NCHUNK, 128, KO, 128).transpose(0, 3, 2, 1)
        )
        in_maps.append(
            {"xc": xc, "wq": wq, "wo": wo, "m2": m2, "m4": m4, "ident": ident}
        )
    return in_maps


def _host_assemble(results):
    y = np.empty((B, N, EMBED), dtype=np.float32)
    for c in range(NCORES):
        bc, q = divmod(c, 4)
        yc = np.asarray(results[c]["y"])  # [2048, 768], rows (blk, j, g)
        part = yc.reshape(4, 4, 128, EMBED).transpose(1, 0, 2, 3)  # [j, blk, g, e]
        y[bc].reshape(4, 4, 4, 128, EMBED)[:, q] = part
    return y


def kernel(x, Wqkv, Wout):
    from concourse.bass_utils import run_bass_kernel_spmd

    if "nc" not in _COMPILED:
        _COMPILED["nc"] = _build_program()
    nc = _COMPILED["nc"]

    in_maps = _host_shard(x, Wqkv, Wout)
    res = run_bass_kernel_spmd(nc, in_maps, core_ids=list(range(NCORES)))
    _COMPILED["last_result"] = res
    return _host_assemble(res.results)


if __name__ == "__main__":
    # smoke build
    nc = _build_program()
    print("built ok; instructions:", len(nc.inst_map))


# revision 33
# speedup vs baseline: 12.3016x; 12.3016x over previous
"""Dilated multi-head attention (nn_DilatedMHA) on 8 trn2 NeuronCores.

Math (reference restructured):
  qkv = x @ Wqkv.T                      [b, n, 3, h, d]   b=2, n=8192, h=12, d=64
  Coupling structure: position i only attends within its mod-2048 class
  {p, p+2048, p+4096, p+6144} (p = i % 2048).  Per group p and head:
    r=1 branch: full 4x4 softmax attention over the 4 slots.
    r=2 branch (p even): 2x2 attention among same-parity slots.
    r=4 branch (p % 4 == 0): adds v.
  out is then normalized by sum over the whole sequence per (b, h*d) channel
  and projected by Wout.

Sharding: core c <- batch c//4, groups p in [(c%4)*512, (c%4)*512+512).
Each core: QKV projection (fp32 matmul), attention on DVE (fp32),
channel-sum AllReduce over the 4 cores of the batch, out-projection
(fp32r matmul, full PE rate) with 1/s folded into Wout.

Precision notes: the normalization denominator min |s| is ~0.018 while the
8192 summands are ~0.5, so upstream per-element noise is amplified ~1000x
into the output.  bf16/tf32 projections fail outright (measured 1.27 / 0.65
abs-max relative error); QKV must run at fp32 matmul precision and all
attention arithmetic in fp32 (DVE is fp32 internally, ACT exp is ~2 ULP).
The channel sums are taken from the exact fp32 PSUM transpose tiles; only
the out-projection operands (downstream of the division) are fp32r.
"""

import sys

if "/opt/trn_rl_repo" not in sys.path:
    sys.path.insert(0, "/opt/trn_rl_repo")

import numpy as np

EMBED = 768
HEADS = 12
HD = 64
B = 2
N = 8192
NCORES = 8
GPC = 512           # groups per core
NBLK = 4            # blocks of 128 groups per core
NCHUNK = 16         # row chunks of 128 per core (blk, j)
KO = 6              # embed // 128

_COMPILED = {}


def _build_program():
    import concourse.mybir as mybir
    import concourse.tile as tile
    from concourse import bacc

    F32 = mybir.dt.float32
    F32R = mybir.dt.float32r
    AX = mybir.AxisListType
    OP = mybir.AluOpType
    ACTF = mybir.ActivationFunctionType

    nc = bacc.Bacc("TRN2", target_bir_lowering=False, debug=False, num_devices=NCORES)

    F16 = mybir.dt.float16

    # --- DRAM I/O ---------------------------------------------------------
    # Q,K projection runs as an fp16 hi/lo split (x = xh + xl exact to
    # ~2^-21): xh@wh + xh@wl + xl@wh at 3 cycles/row vs fp32's 4, with
    # fp32 PSUM accumulation.  Empirically the amplified output error is
    # insensitive to Q/K noise (softmax cancels it) but very sensitive to
    # V noise (V feeds the near-cancelling channel sums), so the V columns
    # keep the full fp32 matmul path.
    xch_d = nc.dram_tensor("xch", [NCHUNK, 128, KO, 128], F16, kind="ExternalInput")
    xcl_d = nc.dram_tensor("xcl", [NCHUNK, 128, KO, 128], F16, kind="ExternalInput")
    xcv_d = nc.dram_tensor("xcv", [NCHUNK, 128, KO, 128], F32, kind="ExternalInput")
    wqh_d = nc.dram_tensor("wqh", [128, KO, 2 * EMBED], F16, kind="ExternalInput")
    wql_d = nc.dram_tensor("wql", [128, KO, 2 * EMBED], F16, kind="ExternalInput")
    wqv_d = nc.dram_tensor("wqv", [128, KO, EMBED], F32, kind="ExternalInput")
    wo_d = nc.dram_tensor("wo", [128, KO, EMBED], F32, kind="ExternalInput")
    m2_d = nc.dram_tensor("m2", [128, 1], F32, kind="ExternalInput")
    m4_d = nc.dram_tensor("m4", [128, 1], F32, kind="ExternalInput")
    id_d = nc.dram_tensor("ident", [128, 128], F32, kind="ExternalInput")
    y_d = nc.dram_tensor("y", [4 * GPC, EMBED], F32, kind="ExternalOutput")

    with tile.TileContext(nc) as tc:
        with (
            tc.tile_pool(name="const", bufs=1) as constp,
            tc.tile_pool(name="oT", bufs=4) as oTp,
            tc.tile_pool(name="mm", bufs=5, space="PSUM") as mmp,
            tc.tile_pool(name="tp", bufs=3, space="PSUM") as tpp,
            tc.tile_pool(name="dram", bufs=2, space="DRAM") as dramp,
        ):
            # --- long-lived SBUF -----------------------------------------
            wqh_sb = constp.tile([128, KO, 2 * EMBED], F16)
            wql_sb = constp.tile([128, KO, 2 * EMBED], F16)
            wqv_sb = constp.tile([128, KO, EMBED], F32)
            wo_sb = constp.tile([128, KO, EMBED], F32)
            m2_sb = constp.tile([128, 1], F32)
            nc.sync.dma_start(m2_sb[:], m2_d[:])
            m4_sb = constp.tile([128, 1], F32)
            nc.sync.dma_start(m4_sb[:], m4_d[:])
            id_sb = constp.tile([128, 128], F32)
            nc.sync.dma_start(id_sb[:], id_d[:])
            s_acc = constp.tile([128, KO], F32)
            nc.vector.memset(s_acc[:], 0.0)

            oT_blocks = []

            def hview(ap2d):
                # [128, 768] -> [128, 12, 64]
                return ap2d.rearrange("p (h d) -> p h d", d=HD)

            # =============== Phase A: QKV + attention ====================
            with (
                tc.tile_pool(name="xc", bufs=2) as xcp,
                tc.tile_pool(name="qkv", bufs=1) as qkvp,
                tc.tile_pool(name="oacc", bufs=1) as oaccp,
                tc.tile_pool(name="att", bufs=1) as attp,
                tc.tile_pool(name="prod", bufs=2) as prodp,
                tc.tile_pool(name="sp", bufs=2) as spp,
            ):
                def load_chunk(chunk):
                    xh_sb = xcp.tile([128, KO, 128], F16, tag="xch")
                    nc.sync.dma_start(xh_sb[:], xch_d[chunk])
                    xl_sb = xcp.tile([128, KO, 128], F16, tag="xcl")
                    nc.sync.dma_start(xl_sb[:], xcl_d[chunk])
                    xv_sb = xcp.tile([128, KO, 128], F32, tag="xcv")
                    nc.sync.dma_start(xv_sb[:], xcv_d[chunk])
                    return xh_sb, xl_sb, xv_sb

                # DMA priority order: first chunk's activations, then weight
                # slices in the order the first matmuls consume them.
                pre = {0: load_chunk(0)}
                for ko in range(KO):
                    nc.sync.dma_start(wqh_sb[:, ko, :], wqh_d[:, ko, :])
                    nc.sync.dma_start(wql_sb[:, ko, :], wql_d[:, ko, :])
                pre[1] = load_chunk(1)
                for ko in range(KO):
                    nc.sync.dma_start(wqv_sb[:, ko, :], wqv_d[:, ko, :])

                for blk in range(NBLK):
                    if blk == 2:
                        # off the critical path at both ends
                        nc.sync.dma_start(wo_sb[:], wo_d[:])
                    Qb = qkvp.tile([128, 4, EMBED], F32, tag="qb")
                    Kb = qkvp.tile([128, 4, EMBED], F32, tag="kb")
                    Vb = qkvp.tile([128, 4, EMBED], F32, tag="vb")
                    dest = {0: Qb, 1: Qb, 2: Kb, 3: Kb, 4: Vb, 5: Vb}
                    for j in range(4):
                        chunk = blk * 4 + j
                        if chunk in pre:
                            xh_sb, xl_sb, xv_sb = pre.pop(chunk)
                        else:
                            xh_sb, xl_sb, xv_sb = load_chunk(chunk)
                        # 6 output column tiles of 384; n outer so tile n's
                        # evacuation overlaps tile n+1's matmuls.
                        for n in range(6):
                            ps = mmp.tile([128, 384], F32, tag="mm")
                            sl = slice(n * 384, (n + 1) * 384)
                            for ko in range(KO):
                                if n < 4:  # Q,K columns: fp16 3-term split
                                    nc.tensor.matmul(
                                        ps[:], lhsT=xh_sb[:, ko, :],
                                        rhs=wqh_sb[:, ko, sl],
                                        start=(ko == 0), stop=False,
                                    )
                                    nc.tensor.matmul(
                                        ps[:], lhsT=xh_sb[:, ko, :],
                                        rhs=wql_sb[:, ko, sl],
                                        start=False, stop=False,
                                    )
                                    nc.tensor.matmul(
                                        ps[:], lhsT=xl_sb[:, ko, :],
                                        rhs=wqh_sb[:, ko, sl],
                                        start=False, stop=(ko == KO - 1),
                                    )
                                else:  # V columns: fp32
                                    nc.tensor.matmul(
                                        ps[:], lhsT=xv_sb[:, ko, :],
                                        rhs=wqv_sb[:, ko, (n - 4) * 384:(n - 3) * 384],
                                        start=(ko == 0), stop=(ko == KO - 1),
                                    )
                            nc.scalar.copy(
                                dest[n][:, j, (n % 2) * 384:(n % 2 + 1) * 384], ps[:]
                            )

                    Q4 = Qb[:].rearrange("p j (h d) -> p j h d", d=HD)
                    K4 = Kb[:].rearrange("p j (h d) -> p j h d", d=HD)
                    V4 = Vb[:].rearrange("p j (h d) -> p j h d", d=HD)

                    # --- scores -> exp'd scores E[p, j, h, j'] ---------------
                    # per-pair so score work starts as soon as each chunk's
                    # QKV lands (ordered by the later-arriving chunk)
                    S = attp.tile([128, 4, HEADS, 4], F32, tag="S")
                    for j, jp in sorted(
                        ((j, jp) for j in range(4) for jp in range(4)),
                        key=lambda t: max(t),
                    ):
                        # late pairs (last chunk) multiply on GPSIMD so the
                        # critical-path DVE only runs the reduces
                        late = max(j, jp) == 3
                        eng = nc.gpsimd if late else nc.vector
                        pr = prodp.tile(
                            [128, HEADS, HD], F32, tag="prodg" if late else "prod"
                        )
                        eng.tensor_mul(pr[:], Q4[:, j], K4[:, jp])
                        nc.vector.reduce_sum(S[:, j, :, jp], pr[:], axis=AX.X)
                    E = attp.tile([128, 4, HEADS, 4], F32, tag="E")
                    nc.scalar.activation(E[:], S[:], ACTF.Exp, scale=0.125)

                    # --- attention weights Wt --------------------------------
                    Z1 = attp.tile([128, 4, HEADS], F32, tag="Z1")
                    nc.vector.reduce_sum(Z1[:], E[:], axis=AX.X)
                    R1 = attp.tile([128, 4, HEADS], F32, tag="R1")
                    nc.vector.reciprocal(R1[:], Z1[:])
                    Z2 = attp.tile([128, 4, HEADS, 2], F32, tag="Z2")
                    nc.vector.tensor_add(Z2[:], E[:, :, :, 0:2], E[:, :, :, 2:4])
                    R2 = attp.tile([128, 4, HEADS, 2], F32, tag="R2")
                    nc.vector.reciprocal(R2[:], Z2[:])

                    W1 = attp.tile([128, 4, HEADS, 4], F32, tag="W1")
                    nc.vector.tensor_mul(
                        W1[:], E[:], R1[:, :, :, None].to_broadcast((128, 4, HEADS, 4))
                    )
                    W2 = attp.tile([128, 4, HEADS, 4], F32, tag="W2")
                    nc.vector.memset(W2[:], 0.0)
                    for par in (0, 1):
                        nc.vector.tensor_mul(
                            W2[:, par::2, :, par::2],
                            E[:, par::2, :, par::2],
                            R2[:, par::2, :, par:par + 1].to_broadcast(
                                (128, 2, HEADS, 2)
                            ),
                        )
                    Wt = attp.tile([128, 4, HEADS, 4], F32, tag="Wt")
                    nc.vector.scalar_tensor_tensor(
                        Wt[:], W2[:], m2_sb[:, 0:1], W1[:], OP.mult, OP.add
                    )
                    for j in range(4):
                        nc.vector.tensor_scalar_add(
                            Wt[:, j, :, j:j + 1], Wt[:, j, :, j:j + 1], m4_sb[:, 0:1]
                        )

                    # --- AV --------------------------------------------------
                    # per-j so oacc[:, j] completes early and its transposes
                    # overlap the remaining rows' AV; rows 2,3 run on GPSIMD
                    # concurrently with rows 0,1 on DVE.
                    oacc = oaccp.tile([128, 4, EMBED], F32, tag="oacc")
                    o4 = oacc[:].rearrange("p j (h d) -> p j h d", d=HD)
                    for j in range(4):
                        eng = nc.vector if j < 2 else nc.gpsimd
                        ptag = "prod" if j < 2 else "prodg"
                        oj = o4[:, j]
                        for jp in range(4):
                            wb = Wt[:, j, :, jp:jp + 1].to_broadcast(
                                (128, HEADS, HD)
                            )
                            if jp == 0:
                                eng.tensor_mul(oj, V4[:, jp], wb)
                            else:
                                pr = prodp.tile([128, HEADS, HD], F32, tag=ptag)
                                eng.tensor_mul(pr[:], V4[:, jp], wb)
                                eng.tensor_add(oj, oj, pr[:])

                    # --- transpose to oT[hd, rows] + channel sums -----------
                    # oT is fp32r (rounded by the ACT copy) for the fp32r
                    # out-projection; the channel sums come from the exact
                    # fp32 PSUM tiles.
                    oT = oTp.tile([128, KO, 4 * 128], F32R, tag="oT")
                    for j in range(4):
                        for ko in range(KO):
                            pt = tpp.tile([128, 128], F32, tag="tp")
                            nc.tensor.transpose(
                                pt[:], oacc[:, j, ko * 128:(ko + 1) * 128], id_sb[:]
                            )
                            nc.scalar.copy(oT[:, ko, j * 128:(j + 1) * 128], pt[:])
                            stmp = spp.tile([128, 1], F32, tag="sp")
                            nc.vector.reduce_sum(stmp[:], pt[:], axis=AX.X)
                            nc.vector.tensor_add(
                                s_acc[:, ko:ko + 1], s_acc[:, ko:ko + 1], stmp[:]
                            )
                    oT_blocks.append(oT)

            # =============== AllReduce of channel sums ===================
            cc_in = dramp.tile([128, KO], F32)
            cc_out = dramp.tile([128, KO], F32)
            nc.gpsimd.dma_start(cc_in[:], s_acc[:])
            nc.gpsimd.collective_compute(
                "AllReduce",
                OP.add,
                replica_groups=[[0, 1, 2, 3], [4, 5, 6, 7]],
                ins=[cc_in[:].opt()],
                outs=[cc_out[:].opt()],
            )
            s_tot = constp.tile([128, KO], F32)
            nc.gpsimd.dma_start(s_tot[:], cc_out[:])
            r_sb = constp.tile([128, KO], F32)
            nc.vector.reciprocal(r_sb[:], s_tot[:])

            # =============== Phase B: out-projection =====================
            with (
                tc.tile_pool(name="ws", bufs=1) as wsp,
                tc.tile_pool(name="fin", bufs=3) as finp,
            ):
                # fold 1/s into Wout rows (per-partition scalar per ko);
                # the DVE write rounds to fp32r as the verifier requires.
                ws_sb = wsp.tile([128, KO, EMBED], F32R)
                for ko in range(KO):
                    nc.vector.tensor_scalar_mul(
                        ws_sb[:, ko, :], wo_sb[:, ko, :], r_sb[:, ko:ko + 1]
                    )

                for blk in range(NBLK):
                    oT = oT_blocks[blk]
                    for rc in range(4):
                        for half in range(2):
                            pf = mmp.tile([128, 384], F32, tag="mm")
                            for ko in range(KO):
                                nc.tensor.matmul(
                                    pf[:],
                                    lhsT=oT[:, ko, rc * 128:(rc + 1) * 128],
                                    rhs=ws_sb[:, ko, half * 384:(half + 1) * 384],
                                    start=(ko == 0),
                                    stop=(ko == KO - 1),
                                )
                            fin = finp.tile([128, 384], F32, tag="fin")
                            nc.scalar.copy(fin[:], pf[:])
                            rows = blk * 512 + rc * 128
                            nc.sync.dma_start(
                                y_d[rows:rows + 128, half * 384:(half + 1) * 384],
                                fin[:],
                            )

    nc.finalize()
    return nc


def _host_shard(x, Wqkv, Wout):
    """Build per-core input maps."""
    x = np.ascontiguousarray(np.asarray(x, dtype=np.float32))
    Wqkv = np.asarray(Wqkv, dtype=np.float32)
    Wout = np.asarray(Wout, dtype=np.float32)

    wq = np.ascontiguousarray(
        Wqkv.T.reshape(KO, 128, 3 * EMBED).transpose(1, 0, 2)
    )
    wqk = wq[:, :, : 2 * EMBED]
    wqh = np.ascontiguousarray(wqk.astype(np.float16))
    wql = np.ascontiguousarray((wqk - wqh.astype(np.float32)).astype(np.float16))
    wqv = np.ascontiguousarray(wq[:, :, 2 * EMBED:])
    wo = np.ascontiguousarray(Wout.T.reshape(KO, 128, EMBED).transpose(1, 0, 2))
    m2 = (np.arange(128) % 2 == 0).astype(np.float32).reshape(128, 1)
    m4 = (np.arange(128) % 4 == 0).astype(np.float32).reshape(128, 1)
    ident = np.eye(128, dtype=np.float32)

    in_maps = []
    for c in range(NCORES):
        bc, q = divmod(c, 4)
        xb = x[bc].reshape(4, 4, 4, 128, EMBED)  # [j, q, blk, g, e]
        mine = xb[:, q]                          # [j, blk, g, e]
        t = np.ascontiguousarray(mine.transpose(1, 0, 2, 3)).reshape(
            NCHUNK, 128, EMBED
        )
        xc = np.ascontiguousarray(
            t.reshape(NCHUNK, 128, KO, 128).transpose(0, 3, 2, 1)
        )
        xch = xc.astype(np.float16)
        xcl = (xc - xch.astype(np.float32)).astype(np.float16)
        in_maps.append(
            {
                "xch": xch, "xcl": xcl, "xcv": xc,
                "wqh": wqh, "wql": wql, "wqv": wqv,
                "wo": wo, "m2": m2, "m4": m4, "ident": ident,
            }
        )
    return in_maps


def _host_assemble(results):
    y = np.empty((B, N, EMBED), dtype=np.float32)
    for c in range(NCORES):
        bc, q = divmod(c, 4)
        yc = np.asarray(results[c]["y"])  # [2048, 768], rows (blk, j, g)
        part = yc.reshape(4, 4, 128, EMBED).transpose(1, 0, 2, 3)  # [j, blk, g, e]
        y[bc].reshape(4, 4, 4, 128, EMBED)[:, q] = part
    return y


def kernel(x, Wqkv, Wout):
    from concourse.bass_utils import run_bass_kernel_spmd

    if "nc" not in _COMPILED:
        _COMPILED["nc"] = _build_program()
    nc = _COMPILED["nc"]

    in_maps = _host_shard(x, Wqkv, Wout)
    res = run_bass_kernel_spmd(nc, in_maps, core_ids=list(range(NCORES)))
    _COMPILED["last_result"] = res
    return _host_assemble(res.results)


if __name__ == "__main__":
    # smoke build
    nc = _build_program()
    print("built ok; instructions:", len(nc.inst_map))
